# revision 20
# baseline (speedup 1.0000x reference)
"""Trainium2 Bass kernel for the autoregressive 2-layer GRU sampler (nn_Prior).

Model (per timestep, batch B=512, hidden H=512, codebook NC=512, T=256):
    x      = emb[tok]
    h1     = GRUCell(x, h1;  W_ih1, W_hh1, b_ih1, b_hh1)
    h2_prev= h1 if t == 0 else h2
    h2     = GRUCell(h1, h2_prev; W_ih2, W_hh2, b_ih2, b_hh2)
    logits = h2 @ W_out.T + b_out
    tok    = categorical(key_t, log_softmax(logits))     # == argmax(logits + gumbel_t)

Strategy:
  - Data-parallel over batch: 8 cores x 64 rows, zero inter-core traffic.
  - The Gumbel noise depends only on the fixed PRNG key 42 => precomputed on
    host (jax CPU, bit-identical to the reference) and streamed per step.
    Sampling on device = argmax(logits + g_t) via DVE max/max_index.
  - The embedding lookup is folded into the layer-1 input projection:
    gi1 = (emb @ W_ih1.T + b)[tok], fetched per step with an indirect-DMA
    row gather from the precomputed E1 table in HBM (no matmul needed).
  - All matmuls in fp32 (PE 4 cycles/row) - float32r only keeps 11 mantissa
    bits which would break the token trajectory vs the fp32 reference.
  - Nonlinearities replicate jax-CPU bitwise: tanh via ScalarE LUT (measured
    bit-exact), sigmoid as 1/(1+exp(-x)) via ScalarE Exp + DVE reciprocal
    (both measured bit-exact vs jax CPU).
  - Weights and per-core state are SBUF-resident for the whole run; per-step
    HBM traffic is only gumbel in (128KB) + logits out (128KB) per core.
"""

import sys

sys.path.insert(0, "/opt/trn_rl_repo")

from contextlib import ExitStack

import numpy as np

B, T_FULL, H, NC = 512, 256, 512, 512
NCORES = 8
BL = B // NCORES  # 64 batch rows per core
KT = H // 128     # 4 contraction k-tiles

_NC_CACHE = {}


def _build_nc(T, with_bias):
    import concourse.bass as bass
    import concourse.mybir as mybir
    from concourse import bacc
    from concourse.tile import TileContext

    F32 = mybir.dt.float32
    U32 = mybir.dt.uint32
    AF = mybir.ActivationFunctionType

    nc = bacc.Bacc("TRN2", target_bir_lowering=False, debug=False,
                   num_devices=NCORES)

    # rhs weight layouts: [128, KT * N] with block kt at [:, kt*N:(kt+1)*N],
    # element [p, kt*N + n] = W.T[kt*128 + p, n]
    E1D = nc.dram_tensor("E1D", [NC, 3 * H], F32, kind="ExternalInput")
    W1H = nc.dram_tensor("W1H", [128, KT * 3 * H], F32, kind="ExternalInput")
    W2I = nc.dram_tensor("W2I", [128, KT * 3 * H], F32, kind="ExternalInput")
    W2H = nc.dram_tensor("W2H", [128, KT * 3 * H], F32, kind="ExternalInput")
    WOT = nc.dram_tensor("WOT", [128, KT * NC], F32, kind="ExternalInput")
    H1_0 = nc.dram_tensor("H1_0", [BL, H], F32, kind="ExternalInput")
    H1T_0 = nc.dram_tensor("H1T_0", [128, KT * BL], F32, kind="ExternalInput")
    IDENT = nc.dram_tensor("IDENT", [BL, BL], F32, kind="ExternalInput")
    G = nc.dram_tensor("G", [T, BL, NC], F32, kind="ExternalInput")
    if with_bias:
        # cols: [hn1 (H) | rz2 (2H) | in2 (H) | hn2 (H)]
        BIASR = nc.dram_tensor("BIASR", [1, 5 * H], F32, kind="ExternalInput")
    OLG = nc.dram_tensor("OLG", [T, BL, NC], F32, kind="ExternalOutput")
    OTOK = nc.dram_tensor("OTOK", [BL, T], U32, kind="ExternalOutput")

    with TileContext(nc) as tc, ExitStack() as ctx:
        wp = ctx.enter_context(tc.tile_pool(name="wp", bufs=1))
        gp = ctx.enter_context(tc.tile_pool(name="gp", bufs=4))
        sp = ctx.enter_context(tc.tile_pool(name="sp", bufs=2))
        hp = ctx.enter_context(tc.tile_pool(name="hp", bufs=2))
        outp = ctx.enter_context(tc.tile_pool(name="outp", bufs=3))
        # 8 PSUM banks: r1 z1 hn1 r2 z2 hn2 (in+lg shared) tr
        ps_r1 = ctx.enter_context(tc.tile_pool(name="ps_r1", bufs=1, space="PSUM"))
        ps_z1 = ctx.enter_context(tc.tile_pool(name="ps_z1", bufs=1, space="PSUM"))
        ps_hn1 = ctx.enter_context(tc.tile_pool(name="ps_hn1", bufs=1, space="PSUM"))
        ps_r2 = ctx.enter_context(tc.tile_pool(name="ps_r2", bufs=1, space="PSUM"))
        ps_z2 = ctx.enter_context(tc.tile_pool(name="ps_z2", bufs=1, space="PSUM"))
        ps_hn2 = ctx.enter_context(tc.tile_pool(name="ps_hn2", bufs=1, space="PSUM"))
        ps_in = ctx.enter_context(tc.tile_pool(name="ps_in", bufs=1, space="PSUM"))
        ps_tr = ctx.enter_context(tc.tile_pool(name="ps_tr", bufs=1, space="PSUM"))

        # ---- load constants ----
        w1h = wp.tile([128, KT * 3 * H], F32, tag="w1h")
        w2i = wp.tile([128, KT * 3 * H], F32, tag="w2i")
        w2h = wp.tile([128, KT * 3 * H], F32, tag="w2h")
        wot = wp.tile([128, KT * NC], F32, tag="wot")
        ident = wp.tile([BL, BL], F32, tag="ident")
        toks = wp.tile([BL, T], U32, tag="toks")
        nc.sync.dma_start(w1h[:], W1H[:])
        nc.sync.dma_start(w2i[:], W2I[:])
        nc.sync.dma_start(w2h[:], W2H[:])
        nc.sync.dma_start(wot[:], WOT[:])
        nc.sync.dma_start(ident[:], IDENT[:])
        if with_bias:
            biasr = wp.tile([1, 5 * H], F32, tag="biasr")
            onesr = wp.tile([1, BL], F32, tag="onesr")
            nc.sync.dma_start(biasr[:], BIASR[:])
            nc.vector.memset(onesr[:], 1.0)

        h1_init = hp.tile([BL, H], F32, tag="h1")
        h1T_init = hp.tile([128, KT * BL], F32, tag="h1T")
        nc.sync.dma_start(h1_init[:], H1_0[:])
        nc.sync.dma_start(h1T_init[:], H1T_0[:])

        # slices of the rz/n column blocks inside a [128, KT*3H] weight tile
        def wslice(w, kt, lo, hi):
            return w[:, kt * 3 * H + lo: kt * 3 * H + hi]

        def transpose_64xH(src, dst_tag):
            """[64, 512] sbuf -> [128, KT*64] sbuf (stationary layout)."""
            tr = ps_tr.tile([128, KT * BL], F32, tag="tr")
            for c in range(KT):
                nc.tensor.transpose(tr[:, c * BL:(c + 1) * BL],
                                    src[:, c * 128:(c + 1) * 128], ident[:],)
            dst = hp.tile([128, KT * BL], F32, tag=dst_tag)
            nc.scalar.copy(dst[:], tr[:])
            return dst

        def gather_e1(idx_col):
            """gi1 = E1[tok] : [BL, 3H] sbuf via indirect DMA row gather."""
            gi = gp.tile([BL, 3 * H], F32, tag="gi")
            nc.gpsimd.indirect_dma_start(
                out=gi[:], out_offset=None, in_=E1D[:],
                in_offset=bass.IndirectOffsetOnAxis(ap=idx_col, axis=0),
            )
            return gi

        def emit_group(ent, groups):
            tile_, started = ent
            for i, (l, r) in enumerate(groups):
                nc.tensor.matmul(tile_[:], l, r,
                                 start=(not started and i == 0),
                                 stop=(i == len(groups) - 1))
            ent[1] = True

        def hslices(hT):
            return [hT[:, kt * BL:(kt + 1) * BL] for kt in range(KT)]

        def emit_l1h(hT):
            """A: layer-1 h-side matmuls into fresh banks1."""
            banks = {"r": [ps_r1.tile([BL, H], F32, tag="b", name="b_r1"), False],
                     "z": [ps_z1.tile([BL, H], F32, tag="b", name="b_z1"), False],
                     "hn": [ps_hn1.tile([BL, H], F32, tag="b", name="b_hn1"), False]}
            hTs = hslices(hT)
            gh = [(hTs[kt], wslice(w1h, kt, 2 * H, 3 * H)) for kt in range(KT)]
            if with_bias:
                gh.append((onesr[:], biasr[:, 0:H]))       # b_hh1 n-part
            emit_group(banks["r"], [(hTs[kt], wslice(w1h, kt, 0, H)) for kt in range(KT)])
            emit_group(banks["z"], [(hTs[kt], wslice(w1h, kt, H, 2 * H)) for kt in range(KT)])
            emit_group(banks["hn"], gh)
            return banks

        def emit_l2h(hT):
            """D: layer-2 h-side matmuls into fresh banks2."""
            banks = {"r": [ps_r2.tile([BL, H], F32, tag="b", name="b_r2"), False],
                     "z": [ps_z2.tile([BL, H], F32, tag="b", name="b_z2"), False],
                     "hn": [ps_hn2.tile([BL, H], F32, tag="b", name="b_hn2"), False]}
            hTs = hslices(hT)
            gr = [(hTs[kt], wslice(w2h, kt, 0, H)) for kt in range(KT)]
            gz = [(hTs[kt], wslice(w2h, kt, H, 2 * H)) for kt in range(KT)]
            gh = [(hTs[kt], wslice(w2h, kt, 2 * H, 3 * H)) for kt in range(KT)]
            if with_bias:
                gr.append((onesr[:], biasr[:, H:2 * H]))       # b2 r
                gz.append((onesr[:], biasr[:, 2 * H:3 * H]))   # b2 z
                gh.append((onesr[:], biasr[:, 4 * H:5 * H]))   # b_hh2 n
            emit_group(banks["r"], gr)
            emit_group(banks["z"], gz)
            emit_group(banks["hn"], gh)
            return banks

        def emit_l2x(banks, xT):
            """layer-2 x-side (h1 @ W_ih2) into banks2 + fresh b_in."""
            xTs = hslices(xT)
            emit_group(banks["r"], [(xTs[kt], wslice(w2i, kt, 0, H)) for kt in range(KT)])
            emit_group(banks["z"], [(xTs[kt], wslice(w2i, kt, H, 2 * H)) for kt in range(KT)])
            gin = [(xTs[kt], wslice(w2i, kt, 2 * H, 3 * H)) for kt in range(KT)]
            if with_bias:
                gin.append((onesr[:], biasr[:, 3 * H:4 * H]))  # b_ih2 n
            bin_ = [ps_in.tile([BL, H], F32, tag="b_in", name="b_in"), False]
            emit_group(bin_, gin)
            banks["in"] = bin_

        def gates(banks, xside, h_prev, htag):
            """GRU gate math. xside: ("gather", gi) or ("mm",).
            Bit-exact vs jax CPU: sigmoid = 1/(1+exp(-x)), tanh via LUT."""
            b_r, b_z, b_hn = banks["r"][0], banks["z"][0], banks["hn"][0]
            # r chain first (rmul depends on it); z chain later ops overlap tanh
            if xside[0] == "gather":
                gi_t = xside[1]
                rzsum = sp.tile([BL, 2 * H], F32, tag="rzsum")
                nc.vector.tensor_add(rzsum[:, 0:H], gi_t[:, 0:H], b_r[:])
                ex = sp.tile([BL, 2 * H], F32, tag="sig_e")
                nc.scalar.activation(ex[:, 0:H], rzsum[:, 0:H], AF.Exp, scale=-1.0)
            else:
                ex = sp.tile([BL, 2 * H], F32, tag="sig_e")
                nc.scalar.activation(ex[:, 0:H], b_r[:], AF.Exp, scale=-1.0)
            y1 = sp.tile([BL, 2 * H], F32, tag="sig_y")
            nc.vector.tensor_scalar_add(y1[:, 0:H], ex[:, 0:H], 1.0)
            rzs = sp.tile([BL, 2 * H], F32, tag="sig_o")
            nc.vector.reciprocal(rzs[:, 0:H], y1[:, 0:H])
            rmul = sp.tile([BL, H], F32, tag="rmul")
            nc.vector.tensor_mul(rmul[:], rzs[:, 0:H], b_hn[:])
            nin = sp.tile([BL, H], F32, tag="nin")
            if xside[0] == "gather":
                nc.vector.tensor_add(nin[:], rmul[:], gi_t[:, 2 * H:3 * H])
            else:
                nc.vector.tensor_add(nin[:], rmul[:], banks["in"][0][:])
            ns = sp.tile([BL, H], F32, tag="ns")
            nc.scalar.activation(ns[:], nin[:], AF.Tanh)
            # z chain (overlaps the tanh on ACT)
            if xside[0] == "gather":
                nc.vector.tensor_add(rzsum[:, H:2 * H], gi_t[:, H:2 * H], b_z[:])
                nc.scalar.activation(ex[:, H:2 * H], rzsum[:, H:2 * H], AF.Exp,
                                     scale=-1.0)
            else:
                nc.scalar.activation(ex[:, H:2 * H], b_z[:], AF.Exp, scale=-1.0)
            nc.vector.tensor_scalar_add(y1[:, H:2 * H], ex[:, H:2 * H], 1.0)
            nc.vector.reciprocal(rzs[:, H:2 * H], y1[:, H:2 * H])
            hmn = sp.tile([BL, H], F32, tag="hmn")
            nc.vector.tensor_sub(hmn[:], h_prev[:], ns[:])
            zm = sp.tile([BL, H], F32, tag="zm")
            nc.vector.tensor_mul(zm[:], rzs[:, H:2 * H], hmn[:])
            hnew = hp.tile([BL, H], F32, tag=htag)
            nc.vector.tensor_add(hnew[:], ns[:], zm[:])
            return hnew

        # ---- prologue: tok0 = 0 ----
        idx0 = wp.tile([BL, 1], U32, tag="idx0")
        nc.vector.memset(idx0[:], 0)
        gi_cur = gather_e1(idx0[:])
        h1_cur, h1T_cur = h1_init, h1T_init
        h2_cur, h2T_cur = None, None

        for t in range(T):
            g = gp.tile([BL, NC], F32, tag="g")
            nc.sync.dma_start(g[:], G[t])

            banks1 = emit_l1h(h1T_cur)
            h1_new = gates(banks1, ("gather", gi_cur), h1_cur, "h1")
            h1T_new = transpose_64xH(h1_new, "h1T")
            banks2 = emit_l2h(h1T_new if t == 0 else h2T_cur)
            emit_l2x(banks2, h1T_new)
            h2p = h1_new if t == 0 else h2_cur
            h2_new = gates(banks2, ("mm",), h2p, "h2")
            h2T_new = transpose_64xH(h2_new, "h2T")

            # ---- logits ----
            lg = ps_in.tile([BL, NC], F32, tag="b_in")
            for kt in range(KT):
                nc.tensor.matmul(lg[:], h2T_new[:, kt * BL:(kt + 1) * BL],
                                 wot[:, kt * NC:(kt + 1) * NC],
                                 start=(kt == 0), stop=(kt == KT - 1))
            lgs = outp.tile([BL, NC], F32, tag="lgs")
            nc.scalar.copy(lgs[:], lg[:])
            nc.sync.dma_start(OLG[t], lgs[:])

            # ---- sample: tok = argmax(logits + g) ----
            y = sp.tile([BL, NC], F32, tag="y")
            nc.vector.tensor_add(y[:], lg[:], g[:])
            mx = sp.tile([BL, 8], F32, tag="mx")
            nc.vector.max(out=mx[:], in_=y[:])
            idx = sp.tile([BL, 8], U32, tag="idx")
            nc.vector.max_index(out=idx[:], in_max=mx[:], in_values=y[:])
            nc.vector.tensor_copy(toks[:, t:t + 1], idx[:, 0:1])
            if t + 1 < T:
                gi_cur = gather_e1(idx[:, 0:1])

            h1_cur, h1T_cur = h1_new, h1T_new
            h2_cur, h2T_cur = h2_new, h2T_new

        nc.sync.dma_start(OTOK[:], toks[:])

    nc.compile()
    return nc


def _host_prep(inputs, T):
    """All host-side constant prep (jax CPU, bit-matching the reference)."""
    import jax
    cpu = jax.devices("cpu")[0]
    import jax.numpy as jnp

    f32 = np.float32
    with jax.default_device(cpu):
        note = jnp.asarray(np.asarray(inputs["note_encoded"], f32))
        genre = jnp.asarray(np.asarray(inputs["genre_cond"], f32))
        W_hid = jnp.asarray(np.asarray(inputs["W_hid"], f32))
        b_hid = jnp.asarray(np.asarray(inputs["b_hid"], f32))
        hidden = jnp.concatenate([note, genre], axis=-1)
        h1_0 = np.asarray(hidden @ W_hid.T + b_hid, f32)          # [B, H]
        emb = jnp.asarray(np.asarray(inputs["emb"], f32))
        W_ih1 = jnp.asarray(np.asarray(inputs["W_ih1"], f32))
        E1 = np.array(emb @ W_ih1.T, f32)                         # [NC, 3H]
        keys = jax.random.split(jax.random.key(42), T_FULL)
        G = np.empty((T, B, NC), f32)
        for t in range(T):
            G[t] = np.asarray(jax.random.gumbel(keys[t], (B, NC), jnp.float32))

    b_ih1 = np.asarray(inputs["b_ih1"], f32)
    b_hh1 = np.asarray(inputs["b_hh1"], f32)
    b_ih2 = np.asarray(inputs["b_ih2"], f32)
    b_hh2 = np.asarray(inputs["b_hh2"], f32)
    b_out = np.asarray(inputs["b_out"], f32)

    E1 = E1 + b_ih1[None, :]
    E1[:, :2 * H] += b_hh1[None, :2 * H]          # fold b_hh1 r,z parts

    with_bias = bool(np.any(b_hh1[2 * H:]) or np.any(b_ih2) or np.any(b_hh2))
    biasrows = np.zeros((1, 5 * H), f32)
    biasrows[0, 0:H] = b_hh1[2 * H:]                        # hn1
    biasrows[0, H:3 * H] = b_ih2[:2 * H] + b_hh2[:2 * H]    # rz2
    biasrows[0, 3 * H:4 * H] = b_ih2[2 * H:]                # in2
    biasrows[0, 4 * H:5 * H] = b_hh2[2 * H:]                # hn2

    if np.any(b_out):
        G = G + b_out[None, None, :].astype(f32)

    def resh(wt, n):   # wt: [H(=K), n] rhs -> [128, KT*n]
        return np.ascontiguousarray(
            wt.reshape(KT, 128, n).transpose(1, 0, 2).reshape(128, KT * n))

    E1D = np.ascontiguousarray(E1)
    W1H = resh(np.ascontiguousarray(np.asarray(inputs["W_hh1"], f32).T), 3 * H)
    W2I = resh(np.ascontiguousarray(np.asarray(inputs["W_ih2"], f32).T), 3 * H)
    W2H = resh(np.ascontiguousarray(np.asarray(inputs["W_hh2"], f32).T), 3 * H)
    WOT = resh(np.ascontiguousarray(np.asarray(inputs["W_out"], f32).T), NC)

    ident = np.eye(BL, dtype=f32)

    return dict(E1D=E1D, W1H=W1H, W2I=W2I, W2H=W2H, WOT=WOT, h1_0=h1_0, G=G,
                ident=ident, with_bias=with_bias,
                biasrows=biasrows, b_out=b_out)


def kernel(T=T_FULL, _trace=False, _tmpdir=None, **inputs):
    from concourse import bass_utils

    prep = _host_prep(inputs, T)
    key = (T, prep["with_bias"])
    if key not in _NC_CACHE:
        _NC_CACHE[key] = _build_nc(T, prep["with_bias"])
    nc = _NC_CACHE[key]

    in_maps = _make_in_maps(prep)

    kw = {}
    if _trace:
        kw = dict(trace=True, tmpdir=_tmpdir)
    res = bass_utils.run_bass_kernel_spmd(nc, in_maps, core_ids=list(range(NCORES)),
                                          **kw)
    if _trace:
        print("exec_time_ns:", res.exec_time_ns)

    code_outs = np.empty((B, T, NC), np.float32)
    pred_outs = np.empty((B, T), np.int32)
    for c in range(NCORES):
        b0 = c * BL
        r = res.results[c]
        code_outs[b0:b0 + BL] = r["OLG"].transpose(1, 0, 2)
        pred_outs[b0:b0 + BL] = r["OTOK"].astype(np.int32)
    if np.any(prep["b_out"]):
        code_outs += prep["b_out"][None, None, :]
    return code_outs, pred_outs


def _make_in_maps(prep):
    in_maps = []
    for c in range(NCORES):
        b0 = c * BL
        h1c = prep["h1_0"][b0:b0 + BL]
        h1Tc = np.ascontiguousarray(
            h1c.T.reshape(KT, 128, BL).transpose(1, 0, 2).reshape(128, KT * BL))
        m = dict(E1D=prep["E1D"], W1H=prep["W1H"], W2I=prep["W2I"],
                 W2H=prep["W2H"], WOT=prep["WOT"],
                 H1_0=np.ascontiguousarray(h1c), H1T_0=h1Tc,
                 IDENT=prep["ident"],
                 G=np.ascontiguousarray(prep["G"][:, b0:b0 + BL, :]))
        if prep["with_bias"]:
            m["BIASR"] = prep["biasrows"]
        in_maps.append(m)
    return in_maps


def bench_exec(T, inputs, iters=6):
    """Time pure device execution (compile cached, inputs device-resident)."""
    import time

    import jax
    from jax.experimental.shard_map import shard_map
    from jax.sharding import Mesh, NamedSharding, PartitionSpec

    import concourse.mybir as mybir
    from concourse import bass2jax
    from concourse.bass2jax import _bass_exec_p, partition_id_tensor

    prep = _host_prep(inputs, T)
    key = (T, prep["with_bias"])
    if key not in _NC_CACHE:
        _NC_CACHE[key] = _build_nc(T, prep["with_bias"])
    nc = _NC_CACHE[key]
    in_maps = _make_in_maps(prep)

    bass2jax.install_neuronx_cc_hook()
    in_names, out_names, out_avals, zero_outs = [], [], [], []
    partition_name = nc.partition_id_tensor.name if nc.partition_id_tensor else None
    for alloc in nc.m.functions[0].allocations:
        if not isinstance(alloc, mybir.MemoryLocationSet):
            continue
        name = alloc.memorylocations[0].name
        if alloc.kind == "ExternalInput":
            if name != partition_name:
                in_names.append(name)
        elif alloc.kind == "ExternalOutput":
            out_avals.append(jax.core.ShapedArray(
                tuple(alloc.tensor_shape), mybir.dt.np(alloc.dtype)))
            out_names.append(name)
            zero_outs.append(np.zeros(alloc.tensor_shape, mybir.dt.np(alloc.dtype)))
    n_params = len(in_names)
    all_in_names = list(in_names) + list(out_names)
    if partition_name is not None:
        all_in_names.append(partition_name)

    def _body(*args):
        operands = list(args)
        if partition_name is not None:
            operands.append(partition_id_tensor())
        return tuple(_bass_exec_p.bind(
            *operands, out_avals=tuple(out_avals), in_names=tuple(all_in_names),
            out_names=tuple(out_names), lowering_input_output_aliases=(),
            sim_require_finite=True, sim_require_nnan=True, nc=nc))

    devices = jax.devices()[:NCORES]
    mesh = Mesh(np.asarray(devices), ("core",))
    spec = PartitionSpec("core")
    sharded = jax.jit(shard_map(_body, mesh=mesh,
                                in_specs=(spec,) * (n_params + len(out_names)),
                                out_specs=(spec,) * len(out_names),
                                check_rep=False), keep_unused=True)
    sh = NamedSharding(mesh, spec)
    concat_in = [jax.device_put(
        np.concatenate([np.asarray(in_maps[c][nm]) for c in range(NCORES)], 0), sh)
        for nm in in_names]
    concat_zeros = [jax.device_put(
        np.zeros((NCORES * z.shape[0], *z.shape[1:]), z.dtype), sh)
        for z in zero_outs]

    times = []
    for i in range(iters):
        t0 = time.perf_counter()
        outs = sharded(*concat_in, *concat_zeros)
        jax.block_until_ready(outs)
        times.append(time.perf_counter() - t0)
    return times, outs, out_names



# revision 22
# speedup vs baseline: 23.0196x; 23.0196x over previous
"""Trainium2 Bass kernel for the autoregressive 2-layer GRU sampler (nn_Prior).

Model (per timestep, batch B=512, hidden H=512, codebook NC=512, T=256):
    x      = emb[tok]
    h1     = GRUCell(x, h1;  W_ih1, W_hh1, b_ih1, b_hh1)
    h2_prev= h1 if t == 0 else h2
    h2     = GRUCell(h1, h2_prev; W_ih2, W_hh2, b_ih2, b_hh2)
    logits = h2 @ W_out.T + b_out
    tok    = categorical(key_t, log_softmax(logits))     # == argmax(logits + gumbel_t)

Strategy:
  - Data-parallel over batch: 8 cores x 64 rows, zero inter-core traffic.
  - The Gumbel noise depends only on the fixed PRNG key 42 => precomputed on
    host (jax CPU, bit-identical to the reference) and streamed per step.
    Sampling on device = argmax(logits + g_t) via DVE max/max_index.
  - The embedding lookup is folded into the layer-1 input projection:
    gi1 = (emb @ W_ih1.T + b)[tok], fetched per step with an indirect-DMA
    row gather from the precomputed E1 table in HBM (no matmul needed).
  - All matmuls in fp32 (PE 4 cycles/row) - float32r only keeps 11 mantissa
    bits which would break the token trajectory vs the fp32 reference.
  - Nonlinearities replicate jax-CPU bitwise: tanh via ScalarE LUT (measured
    bit-exact), sigmoid as 1/(1+exp(-x)) via ScalarE Exp + DVE reciprocal
    (both measured bit-exact vs jax CPU).
  - Weights and per-core state are SBUF-resident for the whole run; per-step
    HBM traffic is only gumbel in (128KB) + logits out (128KB) per core.
"""

import sys

sys.path.insert(0, "/opt/trn_rl_repo")

from contextlib import ExitStack

import numpy as np

B, T_FULL, H, NC = 512, 256, 512, 512
NCORES = 8
BL = B // NCORES  # 64 batch rows per core
KT = H // 128     # 4 contraction k-tiles

_NC_CACHE = {}


def _build_nc(T, with_bias):
    import concourse.bass as bass
    import concourse.mybir as mybir
    from concourse import bacc
    from concourse.tile import TileContext

    F32 = mybir.dt.float32
    U32 = mybir.dt.uint32
    AF = mybir.ActivationFunctionType

    nc = bacc.Bacc("TRN2", target_bir_lowering=False, debug=False,
                   num_devices=NCORES)

    # rhs weight layouts: [128, KT * N] with block kt at [:, kt*N:(kt+1)*N],
    # element [p, kt*N + n] = W.T[kt*128 + p, n]
    E1D = nc.dram_tensor("E1D", [NC, 3 * H], F32, kind="ExternalInput")
    W1H = nc.dram_tensor("W1H", [128, KT * 3 * H], F32, kind="ExternalInput")
    W2I = nc.dram_tensor("W2I", [128, KT * 3 * H], F32, kind="ExternalInput")
    W2H = nc.dram_tensor("W2H", [128, KT * 3 * H], F32, kind="ExternalInput")
    WOT = nc.dram_tensor("WOT", [128, KT * NC], F32, kind="ExternalInput")
    H1_0 = nc.dram_tensor("H1_0", [BL, H], F32, kind="ExternalInput")
    H1T_0 = nc.dram_tensor("H1T_0", [128, KT * BL], F32, kind="ExternalInput")
    IDENT = nc.dram_tensor("IDENT", [BL, BL], F32, kind="ExternalInput")
    G = nc.dram_tensor("G", [T, BL, NC], F32, kind="ExternalInput")
    if with_bias:
        # cols: [hn1 (H) | rz2 (2H) | in2 (H) | hn2 (H)]
        BIASR = nc.dram_tensor("BIASR", [1, 5 * H], F32, kind="ExternalInput")
    OLG = nc.dram_tensor("OLG", [T, BL, NC], F32, kind="ExternalOutput")
    OTOK = nc.dram_tensor("OTOK", [BL, T], U32, kind="ExternalOutput")

    with TileContext(nc) as tc, ExitStack() as ctx:
        wp = ctx.enter_context(tc.tile_pool(name="wp", bufs=1))
        gp = ctx.enter_context(tc.tile_pool(name="gp", bufs=4))
        sp = ctx.enter_context(tc.tile_pool(name="sp", bufs=2))
        hp = ctx.enter_context(tc.tile_pool(name="hp", bufs=2))
        outp = ctx.enter_context(tc.tile_pool(name="outp", bufs=3))
        # 8 PSUM banks: r1 z1 hn1 r2 z2 hn2 (in+lg shared) tr
        ps_r1 = ctx.enter_context(tc.tile_pool(name="ps_r1", bufs=1, space="PSUM"))
        ps_z1 = ctx.enter_context(tc.tile_pool(name="ps_z1", bufs=1, space="PSUM"))
        ps_hn1 = ctx.enter_context(tc.tile_pool(name="ps_hn1", bufs=1, space="PSUM"))
        ps_r2 = ctx.enter_context(tc.tile_pool(name="ps_r2", bufs=1, space="PSUM"))
        ps_z2 = ctx.enter_context(tc.tile_pool(name="ps_z2", bufs=1, space="PSUM"))
        ps_hn2 = ctx.enter_context(tc.tile_pool(name="ps_hn2", bufs=1, space="PSUM"))
        ps_in = ctx.enter_context(tc.tile_pool(name="ps_in", bufs=1, space="PSUM"))
        ps_tr = ctx.enter_context(tc.tile_pool(name="ps_tr", bufs=1, space="PSUM"))

        # ---- load constants ----
        w1h = wp.tile([128, KT * 3 * H], F32, tag="w1h")
        w2i = wp.tile([128, KT * 3 * H], F32, tag="w2i")
        w2h = wp.tile([128, KT * 3 * H], F32, tag="w2h")
        wot = wp.tile([128, KT * NC], F32, tag="wot")
        ident = wp.tile([BL, BL], F32, tag="ident")
        toks = wp.tile([128, T], U32, tag="toks")
        nc.sync.dma_start(w1h[:], W1H[:])
        nc.sync.dma_start(w2i[:], W2I[:])
        nc.sync.dma_start(w2h[:], W2H[:])
        nc.sync.dma_start(wot[:], WOT[:])
        nc.sync.dma_start(ident[:], IDENT[:])
        if with_bias:
            biasr = wp.tile([1, 5 * H], F32, tag="biasr")
            onesr = wp.tile([1, BL], F32, tag="onesr")
            nc.sync.dma_start(biasr[:], BIASR[:])
            nc.vector.memset(onesr[:], 1.0)

        h1_init = hp.tile([BL, H], F32, tag="h1")
        h1T_init = hp.tile([128, KT * BL], F32, tag="h1T")
        nc.sync.dma_start(h1_init[:], H1_0[:])
        nc.sync.dma_start(h1T_init[:], H1T_0[:])

        # slices of the rz/n column blocks inside a [128, KT*3H] weight tile
        def wslice(w, kt, lo, hi):
            return w[:, kt * 3 * H + lo: kt * 3 * H + hi]

        def transpose_64xH(src, dst_tag):
            """[64, 512] sbuf -> [128, KT*64] sbuf (stationary layout)."""
            tr = ps_tr.tile([128, KT * BL], F32, tag="tr")
            for c in range(KT):
                nc.tensor.transpose(tr[:, c * BL:(c + 1) * BL],
                                    src[:, c * 128:(c + 1) * 128], ident[:],)
            dst = hp.tile([128, KT * BL], F32, tag=dst_tag)
            nc.scalar.copy(dst[:], tr[:])
            return dst

        def gather_e1(idx_col):
            """gi1 = E1[tok] : [BL, 3H] sbuf via indirect DMA row gather."""
            gi = gp.tile([BL, 3 * H], F32, tag="gi")
            nc.gpsimd.indirect_dma_start(
                out=gi[:], out_offset=None, in_=E1D[:],
                in_offset=bass.IndirectOffsetOnAxis(ap=idx_col, axis=0),
            )
            return gi

        def emit_group(ent, groups):
            tile_, started = ent
            for i, (l, r) in enumerate(groups):
                nc.tensor.matmul(tile_[:], l, r,
                                 start=(not started and i == 0),
                                 stop=(i == len(groups) - 1))
            ent[1] = True

        def hslices(hT):
            return [hT[:, kt * BL:(kt + 1) * BL] for kt in range(KT)]

        def emit_l1h(hT):
            """A: layer-1 h-side matmuls into fresh banks1."""
            banks = {"r": [ps_r1.tile([BL, H], F32, tag="b", name="b_r1"), False],
                     "z": [ps_z1.tile([BL, H], F32, tag="b", name="b_z1"), False],
                     "hn": [ps_hn1.tile([BL, H], F32, tag="b", name="b_hn1"), False]}
            hTs = hslices(hT)
            gh = [(hTs[kt], wslice(w1h, kt, 2 * H, 3 * H)) for kt in range(KT)]
            if with_bias:
                gh.append((onesr[:], biasr[:, 0:H]))       # b_hh1 n-part
            emit_group(banks["r"], [(hTs[kt], wslice(w1h, kt, 0, H)) for kt in range(KT)])
            emit_group(banks["z"], [(hTs[kt], wslice(w1h, kt, H, 2 * H)) for kt in range(KT)])
            emit_group(banks["hn"], gh)
            return banks

        def emit_l2h(hT):
            """D: layer-2 h-side matmuls into fresh banks2."""
            banks = {"r": [ps_r2.tile([BL, H], F32, tag="b", name="b_r2"), False],
                     "z": [ps_z2.tile([BL, H], F32, tag="b", name="b_z2"), False],
                     "hn": [ps_hn2.tile([BL, H], F32, tag="b", name="b_hn2"), False]}
            hTs = hslices(hT)
            gr = [(hTs[kt], wslice(w2h, kt, 0, H)) for kt in range(KT)]
            gz = [(hTs[kt], wslice(w2h, kt, H, 2 * H)) for kt in range(KT)]
            gh = [(hTs[kt], wslice(w2h, kt, 2 * H, 3 * H)) for kt in range(KT)]
            if with_bias:
                gr.append((onesr[:], biasr[:, H:2 * H]))       # b2 r
                gz.append((onesr[:], biasr[:, 2 * H:3 * H]))   # b2 z
                gh.append((onesr[:], biasr[:, 4 * H:5 * H]))   # b_hh2 n
            emit_group(banks["r"], gr)
            emit_group(banks["z"], gz)
            emit_group(banks["hn"], gh)
            return banks

        def emit_l2x(banks, xT):
            """layer-2 x-side (h1 @ W_ih2) into banks2 + fresh b_in."""
            xTs = hslices(xT)
            emit_group(banks["r"], [(xTs[kt], wslice(w2i, kt, 0, H)) for kt in range(KT)])
            emit_group(banks["z"], [(xTs[kt], wslice(w2i, kt, H, 2 * H)) for kt in range(KT)])
            gin = [(xTs[kt], wslice(w2i, kt, 2 * H, 3 * H)) for kt in range(KT)]
            if with_bias:
                gin.append((onesr[:], biasr[:, 3 * H:4 * H]))  # b_ih2 n
            bin_ = [ps_in.tile([BL, H], F32, tag="b_in", name="b_in"), False]
            emit_group(bin_, gin)
            banks["in"] = bin_

        def gates(banks, xside, h_prev, htag):
            """GRU gate math. xside: ("gather", gi) or ("mm",).
            Bit-exact vs jax CPU: sigmoid = 1/(1+exp(-x)), tanh via LUT."""
            b_r, b_z, b_hn = banks["r"][0], banks["z"][0], banks["hn"][0]
            # r chain first (rmul depends on it); z chain later ops overlap tanh
            if xside[0] == "gather":
                gi_t = xside[1]
                rzsum = sp.tile([BL, 2 * H], F32, tag="rzsum")
                nc.vector.tensor_add(rzsum[:, 0:H], gi_t[:, 0:H], b_r[:])
                ex = sp.tile([BL, 2 * H], F32, tag="sig_e")
                nc.scalar.activation(ex[:, 0:H], rzsum[:, 0:H], AF.Exp, scale=-1.0)
            else:
                ex = sp.tile([BL, 2 * H], F32, tag="sig_e")
                nc.scalar.activation(ex[:, 0:H], b_r[:], AF.Exp, scale=-1.0)
            y1 = sp.tile([BL, 2 * H], F32, tag="sig_y")
            nc.vector.tensor_scalar_add(y1[:, 0:H], ex[:, 0:H], 1.0)
            rzs = sp.tile([BL, 2 * H], F32, tag="sig_o")
            nc.vector.reciprocal(rzs[:, 0:H], y1[:, 0:H])
            rmul = sp.tile([BL, H], F32, tag="rmul")
            nc.vector.tensor_mul(rmul[:], rzs[:, 0:H], b_hn[:])
            nin = sp.tile([BL, H], F32, tag="nin")
            if xside[0] == "gather":
                nc.vector.tensor_add(nin[:], rmul[:], gi_t[:, 2 * H:3 * H])
            else:
                nc.vector.tensor_add(nin[:], rmul[:], banks["in"][0][:])
            ns = sp.tile([BL, H], F32, tag="ns")
            nc.scalar.activation(ns[:], nin[:], AF.Tanh)
            # z chain (overlaps the tanh on ACT)
            if xside[0] == "gather":
                nc.vector.tensor_add(rzsum[:, H:2 * H], gi_t[:, H:2 * H], b_z[:])
                nc.scalar.activation(ex[:, H:2 * H], rzsum[:, H:2 * H], AF.Exp,
                                     scale=-1.0)
            else:
                nc.scalar.activation(ex[:, H:2 * H], b_z[:], AF.Exp, scale=-1.0)
            nc.vector.tensor_scalar_add(y1[:, H:2 * H], ex[:, H:2 * H], 1.0)
            nc.vector.reciprocal(rzs[:, H:2 * H], y1[:, H:2 * H])
            hmn = sp.tile([BL, H], F32, tag="hmn")
            nc.vector.tensor_sub(hmn[:], h_prev[:], ns[:])
            zm = sp.tile([BL, H], F32, tag="zm")
            nc.vector.tensor_mul(zm[:], rzs[:, H:2 * H], hmn[:])
            hnew = hp.tile([BL, H], F32, tag=htag)
            nc.vector.tensor_add(hnew[:], ns[:], zm[:])
            return hnew

        # ---- prologue: tok0 = 0 ----
        idx0 = wp.tile([BL, 1], U32, tag="idx0")
        nc.vector.memset(idx0[:], 0)
        gi_cur = gather_e1(idx0[:])
        h1_cur, h1T_cur = h1_init, h1T_init
        h2_cur, h2T_cur = None, None

        for t in range(T):
            g = gp.tile([128, NC], F32, tag="g")
            nc.sync.dma_start(g[BL:128, :], G[t])

            banks1 = emit_l1h(h1T_cur)
            h1_new = gates(banks1, ("gather", gi_cur), h1_cur, "h1")
            h1T_new = transpose_64xH(h1_new, "h1T")
            banks2 = emit_l2h(h1T_new if t == 0 else h2T_cur)
            emit_l2x(banks2, h1T_new)
            h2p = h1_new if t == 0 else h2_cur
            h2_new = gates(banks2, ("mm",), h2p, "h2")
            h2T_new = transpose_64xH(h2_new, "h2T")

            # ---- logits on partitions 64:128 (PE col-group hi) so the
            # matmuls overlap next-step lo-col matmuls on the real PE ----
            lg = ps_in.tile([128, NC], F32, tag="b_in", name="lg")
            for kt in range(KT):
                nc.tensor.matmul(lg[BL:128, :], h2T_new[:, kt * BL:(kt + 1) * BL],
                                 wot[:, kt * NC:(kt + 1) * NC],
                                 start=(kt == 0), stop=(kt == KT - 1))
            lgs = outp.tile([128, NC], F32, tag="lgs")
            nc.scalar.copy(lgs[BL:128, :], lg[BL:128, :])
            nc.sync.dma_start(OLG[t], lgs[BL:128, :])

            # ---- sample: tok = argmax(logits + g), all on partitions 64:128
            y = sp.tile([128, NC], F32, tag="y")
            nc.vector.tensor_add(y[BL:128, :], lg[BL:128, :], g[BL:128, :])
            mx = sp.tile([128, 8], F32, tag="mx")
            nc.vector.max(out=mx[BL:128, :], in_=y[BL:128, :])
            idx = sp.tile([128, 8], U32, tag="idx")
            nc.vector.max_index(out=idx[BL:128, :], in_max=mx[BL:128, :],
                                in_values=y[BL:128, :])
            nc.vector.tensor_copy(toks[BL:128, t:t + 1], idx[BL:128, 0:1])
            if t + 1 < T:
                idx_lo = sp.tile([BL, 1], U32, tag="idx_lo")
                nc.sync.dma_start(idx_lo[:], idx[BL:128, 0:1])
                gi_cur = gather_e1(idx_lo[:])

            h1_cur, h1T_cur = h1_new, h1T_new
            h2_cur, h2T_cur = h2_new, h2T_new

        nc.sync.dma_start(OTOK[:], toks[BL:128, :])

    nc.compile()
    return nc


def _host_prep(inputs, T):
    """All host-side constant prep (jax CPU, bit-matching the reference)."""
    import jax
    cpu = jax.devices("cpu")[0]
    import jax.numpy as jnp

    f32 = np.float32
    with jax.default_device(cpu):
        note = jnp.asarray(np.asarray(inputs["note_encoded"], f32))
        genre = jnp.asarray(np.asarray(inputs["genre_cond"], f32))
        W_hid = jnp.asarray(np.asarray(inputs["W_hid"], f32))
        b_hid = jnp.asarray(np.asarray(inputs["b_hid"], f32))
        hidden = jnp.concatenate([note, genre], axis=-1)
        h1_0 = np.asarray(hidden @ W_hid.T + b_hid, f32)          # [B, H]
        emb = jnp.asarray(np.asarray(inputs["emb"], f32))
        W_ih1 = jnp.asarray(np.asarray(inputs["W_ih1"], f32))
        E1 = np.array(emb @ W_ih1.T, f32)                         # [NC, 3H]
        keys = jax.random.split(jax.random.key(42), T_FULL)
        G = np.empty((T, B, NC), f32)
        for t in range(T):
            G[t] = np.asarray(jax.random.gumbel(keys[t], (B, NC), jnp.float32))

    b_ih1 = np.asarray(inputs["b_ih1"], f32)
    b_hh1 = np.asarray(inputs["b_hh1"], f32)
    b_ih2 = np.asarray(inputs["b_ih2"], f32)
    b_hh2 = np.asarray(inputs["b_hh2"], f32)
    b_out = np.asarray(inputs["b_out"], f32)

    E1 = E1 + b_ih1[None, :]
    E1[:, :2 * H] += b_hh1[None, :2 * H]          # fold b_hh1 r,z parts

    with_bias = bool(np.any(b_hh1[2 * H:]) or np.any(b_ih2) or np.any(b_hh2))
    biasrows = np.zeros((1, 5 * H), f32)
    biasrows[0, 0:H] = b_hh1[2 * H:]                        # hn1
    biasrows[0, H:3 * H] = b_ih2[:2 * H] + b_hh2[:2 * H]    # rz2
    biasrows[0, 3 * H:4 * H] = b_ih2[2 * H:]                # in2
    biasrows[0, 4 * H:5 * H] = b_hh2[2 * H:]                # hn2

    if np.any(b_out):
        G = G + b_out[None, None, :].astype(f32)

    def resh(wt, n):   # wt: [H(=K), n] rhs -> [128, KT*n]
        return np.ascontiguousarray(
            wt.reshape(KT, 128, n).transpose(1, 0, 2).reshape(128, KT * n))

    E1D = np.ascontiguousarray(E1)
    W1H = resh(np.ascontiguousarray(np.asarray(inputs["W_hh1"], f32).T), 3 * H)
    W2I = resh(np.ascontiguousarray(np.asarray(inputs["W_ih2"], f32).T), 3 * H)
    W2H = resh(np.ascontiguousarray(np.asarray(inputs["W_hh2"], f32).T), 3 * H)
    WOT = resh(np.ascontiguousarray(np.asarray(inputs["W_out"], f32).T), NC)

    ident = np.eye(BL, dtype=f32)

    return dict(E1D=E1D, W1H=W1H, W2I=W2I, W2H=W2H, WOT=WOT, h1_0=h1_0, G=G,
                ident=ident, with_bias=with_bias,
                biasrows=biasrows, b_out=b_out)


def kernel(T=T_FULL, _trace=False, _tmpdir=None, **inputs):
    from concourse import bass_utils

    prep = _host_prep(inputs, T)
    key = (T, prep["with_bias"])
    if key not in _NC_CACHE:
        _NC_CACHE[key] = _build_nc(T, prep["with_bias"])
    nc = _NC_CACHE[key]

    in_maps = _make_in_maps(prep)

    kw = {}
    if _trace:
        kw = dict(trace=True, tmpdir=_tmpdir)
    res = bass_utils.run_bass_kernel_spmd(nc, in_maps, core_ids=list(range(NCORES)),
                                          **kw)
    if _trace:
        print("exec_time_ns:", res.exec_time_ns)

    code_outs = np.empty((B, T, NC), np.float32)
    pred_outs = np.empty((B, T), np.int32)
    for c in range(NCORES):
        b0 = c * BL
        r = res.results[c]
        code_outs[b0:b0 + BL] = r["OLG"].transpose(1, 0, 2)
        pred_outs[b0:b0 + BL] = r["OTOK"].astype(np.int32)
    if np.any(prep["b_out"]):
        code_outs += prep["b_out"][None, None, :]
    return code_outs, pred_outs


def _make_in_maps(prep):
    in_maps = []
    for c in range(NCORES):
        b0 = c * BL
        h1c = prep["h1_0"][b0:b0 + BL]
        h1Tc = np.ascontiguousarray(
            h1c.T.reshape(KT, 128, BL).transpose(1, 0, 2).reshape(128, KT * BL))
        m = dict(E1D=prep["E1D"], W1H=prep["W1H"], W2I=prep["W2I"],
                 W2H=prep["W2H"], WOT=prep["WOT"],
                 H1_0=np.ascontiguousarray(h1c), H1T_0=h1Tc,
                 IDENT=prep["ident"],
                 G=np.ascontiguousarray(prep["G"][:, b0:b0 + BL, :]))
        if prep["with_bias"]:
            m["BIASR"] = prep["biasrows"]
        in_maps.append(m)
    return in_maps


def bench_exec(T, inputs, iters=6):
    """Time pure device execution (compile cached, inputs device-resident)."""
    import time

    import jax
    from jax.experimental.shard_map import shard_map
    from jax.sharding import Mesh, NamedSharding, PartitionSpec

    import concourse.mybir as mybir
    from concourse import bass2jax
    from concourse.bass2jax import _bass_exec_p, partition_id_tensor

    prep = _host_prep(inputs, T)
    key = (T, prep["with_bias"])
    if key not in _NC_CACHE:
        _NC_CACHE[key] = _build_nc(T, prep["with_bias"])
    nc = _NC_CACHE[key]
    in_maps = _make_in_maps(prep)

    bass2jax.install_neuronx_cc_hook()
    in_names, out_names, out_avals, zero_outs = [], [], [], []
    partition_name = nc.partition_id_tensor.name if nc.partition_id_tensor else None
    for alloc in nc.m.functions[0].allocations:
        if not isinstance(alloc, mybir.MemoryLocationSet):
            continue
        name = alloc.memorylocations[0].name
        if alloc.kind == "ExternalInput":
            if name != partition_name:
                in_names.append(name)
        elif alloc.kind == "ExternalOutput":
            out_avals.append(jax.core.ShapedArray(
                tuple(alloc.tensor_shape), mybir.dt.np(alloc.dtype)))
            out_names.append(name)
            zero_outs.append(np.zeros(alloc.tensor_shape, mybir.dt.np(alloc.dtype)))
    n_params = len(in_names)
    all_in_names = list(in_names) + list(out_names)
    if partition_name is not None:
        all_in_names.append(partition_name)

    def _body(*args):
        operands = list(args)
        if partition_name is not None:
            operands.append(partition_id_tensor())
        return tuple(_bass_exec_p.bind(
            *operands, out_avals=tuple(out_avals), in_names=tuple(all_in_names),
            out_names=tuple(out_names), lowering_input_output_aliases=(),
            sim_require_finite=True, sim_require_nnan=True, nc=nc))

    devices = jax.devices()[:NCORES]
    mesh = Mesh(np.asarray(devices), ("core",))
    spec = PartitionSpec("core")
    sharded = jax.jit(shard_map(_body, mesh=mesh,
                                in_specs=(spec,) * (n_params + len(out_names)),
                                out_specs=(spec,) * len(out_names),
                                check_rep=False), keep_unused=True)
    sh = NamedSharding(mesh, spec)
    concat_in = [jax.device_put(
        np.concatenate([np.asarray(in_maps[c][nm]) for c in range(NCORES)], 0), sh)
        for nm in in_names]
    concat_zeros = [jax.device_put(
        np.zeros((NCORES * z.shape[0], *z.shape[1:]), z.dtype), sh)
        for z in zero_outs]

    times = []
    for i in range(iters):
        t0 = time.perf_counter()
        outs = sharded(*concat_in, *concat_zeros)
        jax.block_until_ready(outs)
        times.append(time.perf_counter() - t0)
    return times, outs, out_names



# revision 23
# speedup vs baseline: 23.7581x; 1.0321x over previous
"""Trainium2 Bass kernel for the autoregressive 2-layer GRU sampler (nn_Prior).

Model (per timestep, batch B=512, hidden H=512, codebook NC=512, T=256):
    x      = emb[tok]
    h1     = GRUCell(x, h1;  W_ih1, W_hh1, b_ih1, b_hh1)
    h2_prev= h1 if t == 0 else h2
    h2     = GRUCell(h1, h2_prev; W_ih2, W_hh2, b_ih2, b_hh2)
    logits = h2 @ W_out.T + b_out
    tok    = categorical(key_t, log_softmax(logits))     # == argmax(logits + gumbel_t)

Strategy:
  - Data-parallel over batch: 8 cores x 64 rows, zero inter-core traffic.
  - The Gumbel noise depends only on the fixed PRNG key 42 => precomputed on
    host (jax CPU, bit-identical to the reference) and streamed per step.
    Sampling on device = argmax(logits + g_t) via DVE max/max_index.
  - The embedding lookup is folded into the layer-1 input projection:
    gi1 = (emb @ W_ih1.T + b)[tok], fetched per step with an indirect-DMA
    row gather from the precomputed E1 table in HBM (no matmul needed).
  - All matmuls in fp32 (PE 4 cycles/row) - float32r only keeps 11 mantissa
    bits which would break the token trajectory vs the fp32 reference.
  - Nonlinearities replicate jax-CPU bitwise: tanh via ScalarE LUT (measured
    bit-exact), sigmoid as 1/(1+exp(-x)) via ScalarE Exp + DVE reciprocal
    (both measured bit-exact vs jax CPU).
  - Weights and per-core state are SBUF-resident for the whole run; per-step
    HBM traffic is only gumbel in (128KB) + logits out (128KB) per core.
"""

import sys

sys.path.insert(0, "/opt/trn_rl_repo")

from contextlib import ExitStack

import numpy as np

B, T_FULL, H, NC = 512, 256, 512, 512
NCORES = 8
BL = B // NCORES  # 64 batch rows per core
KT = H // 128     # 4 contraction k-tiles

_NC_CACHE = {}


def _build_nc(T, with_bias):
    import concourse.bass as bass
    import concourse.mybir as mybir
    from concourse import bacc
    from concourse.tile import TileContext

    F32 = mybir.dt.float32
    U32 = mybir.dt.uint32
    AF = mybir.ActivationFunctionType

    nc = bacc.Bacc("TRN2", target_bir_lowering=False, debug=False,
                   num_devices=NCORES)

    # rhs weight layouts: [128, KT * N] with block kt at [:, kt*N:(kt+1)*N],
    # element [p, kt*N + n] = W.T[kt*128 + p, n]
    E1D = nc.dram_tensor("E1D", [NC, 3 * H], F32, kind="ExternalInput")
    W1H = nc.dram_tensor("W1H", [128, KT * 3 * H], F32, kind="ExternalInput")
    W2I = nc.dram_tensor("W2I", [128, KT * 3 * H], F32, kind="ExternalInput")
    W2H = nc.dram_tensor("W2H", [128, KT * 3 * H], F32, kind="ExternalInput")
    WOT = nc.dram_tensor("WOT", [128, KT * NC], F32, kind="ExternalInput")
    H1_0 = nc.dram_tensor("H1_0", [BL, H], F32, kind="ExternalInput")
    H1T_0 = nc.dram_tensor("H1T_0", [128, KT * BL], F32, kind="ExternalInput")
    IDENT = nc.dram_tensor("IDENT", [BL, BL], F32, kind="ExternalInput")
    G = nc.dram_tensor("G", [T, BL, NC], F32, kind="ExternalInput")
    if with_bias:
        # cols: [hn1 (H) | rz2 (2H) | in2 (H) | hn2 (H)]
        BIASR = nc.dram_tensor("BIASR", [1, 5 * H], F32, kind="ExternalInput")
    OLG = nc.dram_tensor("OLG", [T, BL, NC], F32, kind="ExternalOutput")
    OTOK = nc.dram_tensor("OTOK", [BL, T], U32, kind="ExternalOutput")

    with TileContext(nc) as tc, ExitStack() as ctx:
        wp = ctx.enter_context(tc.tile_pool(name="wp", bufs=1))
        gp = ctx.enter_context(tc.tile_pool(name="gp", bufs=4))
        sp = ctx.enter_context(tc.tile_pool(name="sp", bufs=2))
        hp = ctx.enter_context(tc.tile_pool(name="hp", bufs=2))
        outp = ctx.enter_context(tc.tile_pool(name="outp", bufs=3))
        # 8 PSUM banks: r1 z1 hn1 r2 z2 hn2 (in+lg shared) tr
        ps_r1 = ctx.enter_context(tc.tile_pool(name="ps_r1", bufs=1, space="PSUM"))
        ps_z1 = ctx.enter_context(tc.tile_pool(name="ps_z1", bufs=1, space="PSUM"))
        ps_hn1 = ctx.enter_context(tc.tile_pool(name="ps_hn1", bufs=1, space="PSUM"))
        ps_r2 = ctx.enter_context(tc.tile_pool(name="ps_r2", bufs=1, space="PSUM"))
        ps_z2 = ctx.enter_context(tc.tile_pool(name="ps_z2", bufs=1, space="PSUM"))
        ps_hn2 = ctx.enter_context(tc.tile_pool(name="ps_hn2", bufs=1, space="PSUM"))
        ps_in = ctx.enter_context(tc.tile_pool(name="ps_in", bufs=1, space="PSUM"))
        ps_tr = ctx.enter_context(tc.tile_pool(name="ps_tr", bufs=1, space="PSUM"))

        # ---- load constants ----
        w1h = wp.tile([128, KT * 3 * H], F32, tag="w1h")
        w2i = wp.tile([128, KT * 3 * H], F32, tag="w2i")
        w2h = wp.tile([128, KT * 3 * H], F32, tag="w2h")
        wot = wp.tile([128, KT * NC], F32, tag="wot")
        ident = wp.tile([BL, BL], F32, tag="ident")
        toks = wp.tile([BL, T], U32, tag="toks")
        nc.sync.dma_start(w1h[:], W1H[:])
        nc.sync.dma_start(w2i[:], W2I[:])
        nc.sync.dma_start(w2h[:], W2H[:])
        nc.sync.dma_start(wot[:], WOT[:])
        nc.sync.dma_start(ident[:], IDENT[:])
        if with_bias:
            biasr = wp.tile([1, 5 * H], F32, tag="biasr")
            onesr = wp.tile([1, BL], F32, tag="onesr")
            nc.sync.dma_start(biasr[:], BIASR[:])
            nc.vector.memset(onesr[:], 1.0)

        h1_init = hp.tile([BL, H], F32, tag="h1")
        h1T_init = hp.tile([128, KT * BL], F32, tag="h1T")
        nc.sync.dma_start(h1_init[:], H1_0[:])
        nc.sync.dma_start(h1T_init[:], H1T_0[:])

        # slices of the rz/n column blocks inside a [128, KT*3H] weight tile
        def wslice(w, kt, lo, hi):
            return w[:, kt * 3 * H + lo: kt * 3 * H + hi]

        def transpose_64xH(src, dst_tag):
            """[64, 512] sbuf -> [128, KT*64] sbuf (stationary layout)."""
            tr = ps_tr.tile([128, KT * BL], F32, tag="tr")
            for c in range(KT):
                nc.tensor.transpose(tr[:, c * BL:(c + 1) * BL],
                                    src[:, c * 128:(c + 1) * 128], ident[:],)
            dst = hp.tile([128, KT * BL], F32, tag=dst_tag)
            nc.scalar.copy(dst[:], tr[:])
            return dst

        def gather_e1(idx_col):
            """gi1 = E1[tok] : [BL, 3H] sbuf via indirect DMA row gather."""
            gi = gp.tile([BL, 3 * H], F32, tag="gi")
            nc.gpsimd.indirect_dma_start(
                out=gi[:], out_offset=None, in_=E1D[:],
                in_offset=bass.IndirectOffsetOnAxis(ap=idx_col, axis=0),
            )
            return gi

        def emit_group(ent, groups):
            tile_, started = ent
            for i, (l, r) in enumerate(groups):
                nc.tensor.matmul(tile_[:], l, r,
                                 start=(not started and i == 0),
                                 stop=(i == len(groups) - 1))
            ent[1] = True

        def hslices(hT):
            return [hT[:, kt * BL:(kt + 1) * BL] for kt in range(KT)]

        def emit_l1h(hT):
            """A: layer-1 h-side matmuls into fresh banks1."""
            banks = {"r": [ps_r1.tile([BL, H], F32, tag="b", name="b_r1"), False],
                     "z": [ps_z1.tile([BL, H], F32, tag="b", name="b_z1"), False],
                     "hn": [ps_hn1.tile([BL, H], F32, tag="b", name="b_hn1"), False]}
            hTs = hslices(hT)
            gh = [(hTs[kt], wslice(w1h, kt, 2 * H, 3 * H)) for kt in range(KT)]
            if with_bias:
                gh.append((onesr[:], biasr[:, 0:H]))       # b_hh1 n-part
            emit_group(banks["r"], [(hTs[kt], wslice(w1h, kt, 0, H)) for kt in range(KT)])
            emit_group(banks["z"], [(hTs[kt], wslice(w1h, kt, H, 2 * H)) for kt in range(KT)])
            emit_group(banks["hn"], gh)
            return banks

        def emit_l2h(hT):
            """D: layer-2 h-side matmuls into fresh banks2."""
            banks = {"r": [ps_r2.tile([BL, H], F32, tag="b", name="b_r2"), False],
                     "z": [ps_z2.tile([BL, H], F32, tag="b", name="b_z2"), False],
                     "hn": [ps_hn2.tile([BL, H], F32, tag="b", name="b_hn2"), False]}
            hTs = hslices(hT)
            gr = [(hTs[kt], wslice(w2h, kt, 0, H)) for kt in range(KT)]
            gz = [(hTs[kt], wslice(w2h, kt, H, 2 * H)) for kt in range(KT)]
            gh = [(hTs[kt], wslice(w2h, kt, 2 * H, 3 * H)) for kt in range(KT)]
            if with_bias:
                gr.append((onesr[:], biasr[:, H:2 * H]))       # b2 r
                gz.append((onesr[:], biasr[:, 2 * H:3 * H]))   # b2 z
                gh.append((onesr[:], biasr[:, 4 * H:5 * H]))   # b_hh2 n
            emit_group(banks["r"], gr)
            emit_group(banks["z"], gz)
            emit_group(banks["hn"], gh)
            return banks

        def emit_l2x(banks, xT):
            """layer-2 x-side (h1 @ W_ih2) into banks2 + fresh b_in."""
            xTs = hslices(xT)
            emit_group(banks["r"], [(xTs[kt], wslice(w2i, kt, 0, H)) for kt in range(KT)])
            emit_group(banks["z"], [(xTs[kt], wslice(w2i, kt, H, 2 * H)) for kt in range(KT)])
            gin = [(xTs[kt], wslice(w2i, kt, 2 * H, 3 * H)) for kt in range(KT)]
            if with_bias:
                gin.append((onesr[:], biasr[:, 3 * H:4 * H]))  # b_ih2 n
            bin_ = [ps_in.tile([BL, H], F32, tag="b_in", name="b_in"), False]
            emit_group(bin_, gin)
            banks["in"] = bin_

        def gates(banks, xside, h_prev, htag):
            """GRU gate math. xside: ("gather", gi) or ("mm",).
            Bit-exact vs jax CPU: sigmoid = 1/(1+exp(-x)), tanh via LUT."""
            b_r, b_z, b_hn = banks["r"][0], banks["z"][0], banks["hn"][0]
            # r chain first (rmul depends on it); z chain later ops overlap tanh
            if xside[0] == "gather":
                gi_t = xside[1]
                rzsum = sp.tile([BL, 2 * H], F32, tag="rzsum")
                nc.vector.tensor_add(rzsum[:, 0:H], gi_t[:, 0:H], b_r[:])
                ex = sp.tile([BL, 2 * H], F32, tag="sig_e")
                nc.scalar.activation(ex[:, 0:H], rzsum[:, 0:H], AF.Exp, scale=-1.0)
            else:
                ex = sp.tile([BL, 2 * H], F32, tag="sig_e")
                nc.scalar.activation(ex[:, 0:H], b_r[:], AF.Exp, scale=-1.0)
            y1 = sp.tile([BL, 2 * H], F32, tag="sig_y")
            nc.vector.tensor_scalar_add(y1[:, 0:H], ex[:, 0:H], 1.0)
            rzs = sp.tile([BL, 2 * H], F32, tag="sig_o")
            nc.vector.reciprocal(rzs[:, 0:H], y1[:, 0:H])
            rmul = sp.tile([BL, H], F32, tag="rmul")
            nc.vector.tensor_mul(rmul[:], rzs[:, 0:H], b_hn[:])
            nin = sp.tile([BL, H], F32, tag="nin")
            if xside[0] == "gather":
                nc.vector.tensor_add(nin[:], rmul[:], gi_t[:, 2 * H:3 * H])
            else:
                nc.vector.tensor_add(nin[:], rmul[:], banks["in"][0][:])
            ns = sp.tile([BL, H], F32, tag="ns")
            nc.scalar.activation(ns[:], nin[:], AF.Tanh)
            # z chain (overlaps the tanh on ACT)
            if xside[0] == "gather":
                nc.vector.tensor_add(rzsum[:, H:2 * H], gi_t[:, H:2 * H], b_z[:])
                nc.scalar.activation(ex[:, H:2 * H], rzsum[:, H:2 * H], AF.Exp,
                                     scale=-1.0)
            else:
                nc.scalar.activation(ex[:, H:2 * H], b_z[:], AF.Exp, scale=-1.0)
            nc.vector.tensor_scalar_add(y1[:, H:2 * H], ex[:, H:2 * H], 1.0)
            nc.vector.reciprocal(rzs[:, H:2 * H], y1[:, H:2 * H])
            hmn = sp.tile([BL, H], F32, tag="hmn")
            nc.vector.tensor_sub(hmn[:], h_prev[:], ns[:])
            zm = sp.tile([BL, H], F32, tag="zm")
            nc.vector.tensor_mul(zm[:], rzs[:, H:2 * H], hmn[:])
            hnew = hp.tile([BL, H], F32, tag=htag)
            nc.vector.tensor_add(hnew[:], ns[:], zm[:])
            return hnew

        # ---- prologue: tok0 = 0 ----
        idx0 = wp.tile([BL, 1], U32, tag="idx0")
        nc.vector.memset(idx0[:], 0)
        gi_cur = gather_e1(idx0[:])
        h1_cur, h1T_cur = h1_init, h1T_init
        h2_cur, h2T_cur = None, None

        for t in range(T):
            g = gp.tile([BL, NC], F32, tag="g")
            nc.sync.dma_start(g[:], G[t])

            banks1 = emit_l1h(h1T_cur)
            h1_new = gates(banks1, ("gather", gi_cur), h1_cur, "h1")
            h1T_new = transpose_64xH(h1_new, "h1T")
            banks2 = emit_l2h(h1T_new if t == 0 else h2T_cur)
            emit_l2x(banks2, h1T_new)
            h2p = h1_new if t == 0 else h2_cur
            h2_new = gates(banks2, ("mm",), h2p, "h2")
            h2T_new = transpose_64xH(h2_new, "h2T")

            # ---- logits ----
            lg = ps_in.tile([BL, NC], F32, tag="b_in")
            for kt in range(KT):
                nc.tensor.matmul(lg[:], h2T_new[:, kt * BL:(kt + 1) * BL],
                                 wot[:, kt * NC:(kt + 1) * NC],
                                 start=(kt == 0), stop=(kt == KT - 1))
            lgs = outp.tile([BL, NC], F32, tag="lgs")
            nc.scalar.copy(lgs[:], lg[:])
            nc.sync.dma_start(OLG[t], lgs[:])

            # ---- sample: tok = argmax(logits + g) ----
            y = sp.tile([BL, NC], F32, tag="y")
            nc.vector.tensor_add(y[:], lg[:], g[:])
            mx = sp.tile([BL, 8], F32, tag="mx")
            nc.vector.max(out=mx[:], in_=y[:])
            idx = sp.tile([BL, 8], U32, tag="idx")
            nc.vector.max_index(out=idx[:], in_max=mx[:], in_values=y[:])
            nc.vector.tensor_copy(toks[:, t:t + 1], idx[:, 0:1])
            if t + 1 < T:
                gi_cur = gather_e1(idx[:, 0:1])

            h1_cur, h1T_cur = h1_new, h1T_new
            h2_cur, h2T_cur = h2_new, h2T_new

        nc.sync.dma_start(OTOK[:], toks[:])

    nc.compile()
    return nc


def _host_prep(inputs, T):
    """All host-side constant prep (jax CPU, bit-matching the reference)."""
    import jax
    cpu = jax.devices("cpu")[0]
    import jax.numpy as jnp

    f32 = np.float32
    with jax.default_device(cpu):
        note = jnp.asarray(np.asarray(inputs["note_encoded"], f32))
        genre = jnp.asarray(np.asarray(inputs["genre_cond"], f32))
        W_hid = jnp.asarray(np.asarray(inputs["W_hid"], f32))
        b_hid = jnp.asarray(np.asarray(inputs["b_hid"], f32))
        hidden = jnp.concatenate([note, genre], axis=-1)
        h1_0 = np.asarray(hidden @ W_hid.T + b_hid, f32)          # [B, H]
        emb = jnp.asarray(np.asarray(inputs["emb"], f32))
        W_ih1 = jnp.asarray(np.asarray(inputs["W_ih1"], f32))
        E1 = np.array(emb @ W_ih1.T, f32)                         # [NC, 3H]
        keys = jax.random.split(jax.random.key(42), T_FULL)
        G = np.empty((T, B, NC), f32)
        for t in range(T):
            G[t] = np.asarray(jax.random.gumbel(keys[t], (B, NC), jnp.float32))

    b_ih1 = np.asarray(inputs["b_ih1"], f32)
    b_hh1 = np.asarray(inputs["b_hh1"], f32)
    b_ih2 = np.asarray(inputs["b_ih2"], f32)
    b_hh2 = np.asarray(inputs["b_hh2"], f32)
    b_out = np.asarray(inputs["b_out"], f32)

    E1 = E1 + b_ih1[None, :]
    E1[:, :2 * H] += b_hh1[None, :2 * H]          # fold b_hh1 r,z parts

    with_bias = bool(np.any(b_hh1[2 * H:]) or np.any(b_ih2) or np.any(b_hh2))
    biasrows = np.zeros((1, 5 * H), f32)
    biasrows[0, 0:H] = b_hh1[2 * H:]                        # hn1
    biasrows[0, H:3 * H] = b_ih2[:2 * H] + b_hh2[:2 * H]    # rz2
    biasrows[0, 3 * H:4 * H] = b_ih2[2 * H:]                # in2
    biasrows[0, 4 * H:5 * H] = b_hh2[2 * H:]                # hn2

    if np.any(b_out):
        G = G + b_out[None, None, :].astype(f32)

    def resh(wt, n):   # wt: [H(=K), n] rhs -> [128, KT*n]
        return np.ascontiguousarray(
            wt.reshape(KT, 128, n).transpose(1, 0, 2).reshape(128, KT * n))

    E1D = np.ascontiguousarray(E1)
    W1H = resh(np.ascontiguousarray(np.asarray(inputs["W_hh1"], f32).T), 3 * H)
    W2I = resh(np.ascontiguousarray(np.asarray(inputs["W_ih2"], f32).T), 3 * H)
    W2H = resh(np.ascontiguousarray(np.asarray(inputs["W_hh2"], f32).T), 3 * H)
    WOT = resh(np.ascontiguousarray(np.asarray(inputs["W_out"], f32).T), NC)

    ident = np.eye(BL, dtype=f32)

    return dict(E1D=E1D, W1H=W1H, W2I=W2I, W2H=W2H, WOT=WOT, h1_0=h1_0, G=G,
                ident=ident, with_bias=with_bias,
                biasrows=biasrows, b_out=b_out)


def kernel(T=T_FULL, _trace=False, _tmpdir=None, **inputs):
    from concourse import bass_utils

    prep = _host_prep(inputs, T)
    key = (T, prep["with_bias"])
    if key not in _NC_CACHE:
        _NC_CACHE[key] = _build_nc(T, prep["with_bias"])
    nc = _NC_CACHE[key]

    in_maps = _make_in_maps(prep)

    kw = {}
    if _trace:
        kw = dict(trace=True, tmpdir=_tmpdir)
    res = bass_utils.run_bass_kernel_spmd(nc, in_maps, core_ids=list(range(NCORES)),
                                          **kw)
    if _trace:
        print("exec_time_ns:", res.exec_time_ns)

    code_outs = np.empty((B, T, NC), np.float32)
    pred_outs = np.empty((B, T), np.int32)
    for c in range(NCORES):
        b0 = c * BL
        r = res.results[c]
        code_outs[b0:b0 + BL] = r["OLG"].transpose(1, 0, 2)
        pred_outs[b0:b0 + BL] = r["OTOK"].astype(np.int32)
    if np.any(prep["b_out"]):
        code_outs += prep["b_out"][None, None, :]
    return code_outs, pred_outs


def _make_in_maps(prep):
    in_maps = []
    for c in range(NCORES):
        b0 = c * BL
        h1c = prep["h1_0"][b0:b0 + BL]
        h1Tc = np.ascontiguousarray(
            h1c.T.reshape(KT, 128, BL).transpose(1, 0, 2).reshape(128, KT * BL))
        m = dict(E1D=prep["E1D"], W1H=prep["W1H"], W2I=prep["W2I"],
                 W2H=prep["W2H"], WOT=prep["WOT"],
                 H1_0=np.ascontiguousarray(h1c), H1T_0=h1Tc,
                 IDENT=prep["ident"],
                 G=np.ascontiguousarray(prep["G"][:, b0:b0 + BL, :]))
        if prep["with_bias"]:
            m["BIASR"] = prep["biasrows"]
        in_maps.append(m)
    return in_maps


def bench_exec(T, inputs, iters=6):
    """Time pure device execution (compile cached, inputs device-resident)."""
    import time

    import jax
    from jax.experimental.shard_map import shard_map
    from jax.sharding import Mesh, NamedSharding, PartitionSpec

    import concourse.mybir as mybir
    from concourse import bass2jax
    from concourse.bass2jax import _bass_exec_p, partition_id_tensor

    prep = _host_prep(inputs, T)
    key = (T, prep["with_bias"])
    if key not in _NC_CACHE:
        _NC_CACHE[key] = _build_nc(T, prep["with_bias"])
    nc = _NC_CACHE[key]
    in_maps = _make_in_maps(prep)

    bass2jax.install_neuronx_cc_hook()
    in_names, out_names, out_avals, zero_outs = [], [], [], []
    partition_name = nc.partition_id_tensor.name if nc.partition_id_tensor else None
    for alloc in nc.m.functions[0].allocations:
        if not isinstance(alloc, mybir.MemoryLocationSet):
            continue
        name = alloc.memorylocations[0].name
        if alloc.kind == "ExternalInput":
            if name != partition_name:
                in_names.append(name)
        elif alloc.kind == "ExternalOutput":
            out_avals.append(jax.core.ShapedArray(
                tuple(alloc.tensor_shape), mybir.dt.np(alloc.dtype)))
            out_names.append(name)
            zero_outs.append(np.zeros(alloc.tensor_shape, mybir.dt.np(alloc.dtype)))
    n_params = len(in_names)
    all_in_names = list(in_names) + list(out_names)
    if partition_name is not None:
        all_in_names.append(partition_name)

    def _body(*args):
        operands = list(args)
        if partition_name is not None:
            operands.append(partition_id_tensor())
        return tuple(_bass_exec_p.bind(
            *operands, out_avals=tuple(out_avals), in_names=tuple(all_in_names),
            out_names=tuple(out_names), lowering_input_output_aliases=(),
            sim_require_finite=True, sim_require_nnan=True, nc=nc))

    devices = jax.devices()[:NCORES]
    mesh = Mesh(np.asarray(devices), ("core",))
    spec = PartitionSpec("core")
    sharded = jax.jit(shard_map(_body, mesh=mesh,
                                in_specs=(spec,) * (n_params + len(out_names)),
                                out_specs=(spec,) * len(out_names),
                                check_rep=False), keep_unused=True)
    sh = NamedSharding(mesh, spec)
    concat_in = [jax.device_put(
        np.concatenate([np.asarray(in_maps[c][nm]) for c in range(NCORES)], 0), sh)
        for nm in in_names]
    concat_zeros = [jax.device_put(
        np.zeros((NCORES * z.shape[0], *z.shape[1:]), z.dtype), sh)
        for z in zero_outs]

    times = []
    for i in range(iters):
        t0 = time.perf_counter()
        outs = sharded(*concat_in, *concat_zeros)
        jax.block_until_ready(outs)
        times.append(time.perf_counter() - t0)
    return times, outs, out_names

